# revision 2
# baseline (speedup 1.0000x reference)
"""Self-contained Trainium2 Bass kernel for per-batch out = X @ (X^T @ X).

Full input: [8, 4096, 512] fp32. Sharding: data-parallel over batch --
core b computes batch element b entirely on its own NeuronCore
(no cross-core communication).

Per-core algorithm (X is [4096, 512], S=4096, D=512), bf16 compute
(inputs are N(0,1); bf16 operands with fp32 PSUM accumulation give
~2e-3 relative error, far inside the 2e-2 gate), fp32 output:

  Cast:    X tiles are DMA'd in as fp32 (GpSimd-issued so input issue
           never queues behind the sync engine's output-DMA waits) and
           cast to bf16 by DVE, one 128-row block at a time. All 8 X
           tiles stay resident so the next rep's casts run during this
           rep's phase 2.
  Phase 1: G = X^T @ X -- G is symmetric, so matmuls compute only the
           upper-triangle block-rows: row m covers columns [128m:512]
           (N = 512/384/256/128), contracting over S in 32 k-steps
           into 4 PSUM banks (fp32), ACT-copied to one SBUF row tile
           per block-row as bf16 (separate tiles keep phase-2 deps
           per-row so accumulation starts as soon as row 0 lands).
  Fixup:   the 6 lower-triangle blocks G[n, 128m:128(m+1)] (n>m) are
           PE-transposes of the computed G[m, 128n:128(n+1)].
  X^T:     d-blocks 0,1 are built via 64 PE transposes (128x128 bf16,
           1 cycle/row), 2 per PSUM bank, ACT-copied to SBUF; d-blocks
           2,3 are built by DVE StreamTranspose: 16 instructions, one
           per (output partition group a, input partition group b)
           pair, each transposing every 32x32 block with that (a, b)
           across all 32 k-tiles in one go (StreamTranspose pairs
           blocks positionally between the in/out 2D views, so a fixed
           partition-group pair per instruction is required; wide free
           dims amortize the instruction overhead). This splits the
           transpose load across PE and the otherwise-idle DVE.
  Phase 2: out = X @ G -- stationary operand is an X^T tile (bf16),
           moving operand is a G block-row [128, 512] bf16,
           accumulating fp32 over 4 d-blocks; one 512-wide ACT copy
           PSUM->SBUF, then DMA out (fp32) on the sync queue.

  xt and the G row tiles are double-buffered by rep parity so rep r+1's
  phase 1 never waits on rep r's phase 2.

Known stack hazards worked around here:
  - plain fp32 matmul hangs on HW -> f32r/bf16 operands only.
  - DVE reading more than 512 bytes/partition from PSUM hangs ->
    wide fp32 PSUM reads go to ACT; DVE PSUM reads stay at
    256 bf16 elements (512 B) per instruction.
  - DMA cannot cast -> DRAM inputs are declared float32r (same 4-byte
    layout as fp32; numpy float32 binds unchanged) and cast on-chip.
"""

import sys

sys.path.insert(0, "/opt/trn_rl_repo")

import numpy as np  # noqa: E402
import concourse.bacc as bacc  # noqa: E402
import concourse.mybir as mybir  # noqa: E402
import concourse.tile as tile  # noqa: E402
from concourse.bass_utils import run_bass_kernel_spmd  # noqa: E402

B, S, D = 8, 4096, 512
P = 128
ST = S // P  # 32 s-tiles
DT = D // P  # 4 d-tiles
SG = 4  # s-tiles per input DMA group
F32 = mybir.dt.float32
F32R = mybir.dt.float32r
BF16 = mybir.dt.bfloat16

# Phase-1 (gram) matmul m-block row m covers columns [128m:512].
G_START = [0 * P, 1 * P, 2 * P, 3 * P]
# Lower-triangle blocks reconstructed by transpose: (src_row, dst_row).
G_FIX = [(0, 1), (0, 2), (1, 2), (0, 3), (1, 3), (2, 3)]

_cache: dict = {}


def _build(reps=1):
    nc = bacc.Bacc("TRN2", target_bir_lowering=False, debug=False)
    x = nc.dram_tensor("x", [S, D], F32R, kind="ExternalInput")
    ident = nc.dram_tensor("ident", [P, P], F32R, kind="ExternalInput")
    out = nc.dram_tensor("out", [S, D], F32, kind="ExternalOutput")

    with tile.TileContext(nc) as tc:
        with (
            tc.tile_pool(name="xs", bufs=3) as xs_pool,
            tc.tile_pool(name="persist", bufs=1) as persist,
            tc.tile_pool(name="osb", bufs=4) as osb_pool,
            tc.tile_pool(name="gps", bufs=DT, space="PSUM") as gps_pool,
            tc.tile_pool(name="tps", bufs=2, space="PSUM") as tps_pool,
            tc.tile_pool(name="ops", bufs=2, space="PSUM") as ops_pool,
        ):
            idt = persist.tile([P, P], F32R, tag="ident", name="idt")
            nc.sync.dma_start(idt[:], ident[:])
            idt_b = persist.tile([P, P], BF16, tag="identb", name="idtb")
            nc.vector.tensor_copy(idt_b[:], idt[:])

            # Double-buffered by rep parity:
            # xt[p, m, s] = x[s, m*128 + p] (bf16)
            # g_row[m][p, e] = gram[m*128 + p, e] (bf16)
            xts = [
                persist.tile([P, DT, S], BF16, tag=f"xt{par}", name=f"xt{par}")
                for par in range(2)
            ]
            # xbs[par][p, k, d] = bf16 x[128k + p, d]: one tile per parity so
            # the DVE StreamTranspose free dims can span all 32 k-tiles.
            xbs = [
                persist.tile([P, ST, D], BF16, tag=f"xb{par}", name=f"xb{par}")
                for par in range(2)
            ]
            g_rows = [
                [
                    persist.tile([P, D], BF16, tag=f"g{par}_{m}", name=f"g{par}_{m}")
                    for m in range(DT)
                ]
                for par in range(2)
            ]

            for rep in range(reps):
                xt = xts[rep % 2]
                xb = xbs[rep % 2]
                g_row = g_rows[rep % 2]

                xg = []
                for j in range(ST // SG):
                    t = xs_pool.tile([P, SG, D], F32R, tag="x", name=f"xg{rep}_{j}")
                    nc.gpsimd.dma_start(
                        t[:],
                        x.rearrange("(p r) d -> p r d", p=P)[
                            :, j * SG : (j + 1) * SG, :
                        ],
                    )
                    xg.append(t)

                for j in range(ST // SG):
                    for n in range(SG):
                        k = j * SG + n
                        if k % 8 < 3:
                            nc.scalar.copy(xb[:, k, :], xg[j][:, n, :])
                        else:
                            nc.vector.tensor_copy(xb[:, k, :], xg[j][:, n, :])

                # DVE StreamTranspose for d-blocks 2,3 of X^T: instruction
                # (m, a, b) transposes every 32x32 block whose output
                # partition group is a and input partition group is b,
                # across all 32 k-tiles at once. Views are (32, k, 32) on
                # both sides so the positional block pairing lines up
                # (StreamTranspose cannot move blocks across partition
                # groups within one instruction).
                xt_k = xt.rearrange("p m (k v) -> p m k v", v=P)
                for m in range(3, 4):
                    for a in range(4):
                        for b in range(4):
                            nc.vector.transpose(
                                xt_k[32 * a : 32 * (a + 1), m, :, 32 * b : 32 * (b + 1)],
                                xb[
                                    32 * b : 32 * (b + 1),
                                    :,
                                    m * P + 32 * a : m * P + 32 * (a + 1),
                                ],
                            )
                # m=2, even k on DVE as well (odd k stays on PE)
                xt_e = xt.rearrange("p m (k e v) -> p m k e v", e=2, v=P)
                xb_e = xb.rearrange("p (k e) d -> p k e d", e=2)
                for a in range(4):
                    for b in range(4):
                        nc.vector.transpose(
                            xt_e[32 * a : 32 * (a + 1), 2, :, 0, 32 * b : 32 * (b + 1)],
                            xb_e[
                                32 * b : 32 * (b + 1),
                                :,
                                0,
                                2 * P + 32 * a : 2 * P + 32 * (a + 1),
                            ],
                        )

                def xs(k):
                    return xb[:, k, :]

                g_ps = [
                    gps_pool.tile(
                        [P, D - G_START[m]], F32, tag="g", name=f"gps{rep}_{m}"
                    )
                    for m in range(DT)
                ]
                # Gram runs m-major: finish G block-row 0 first so its SBUF
                # copy and the fixup transposes it feeds happen while PE is
                # still on rows 1-3 -- by phase 2 every G row is ready and
                # the phase boundary has no copy/fixup bubble. The X^T
                # transpose k-groups are spread across the row sections to
                # pace the PSUM ring against the ACT copy stream.
                for m in range(DT):
                    for k in range(ST):
                        nc.tensor.matmul(
                            g_ps[m][:],
                            xs(k)[:, m * P : (m + 1) * P],
                            xs(k)[:, G_START[m] :],
                            start=(k == 0),
                            stop=(k == ST - 1),
                        )
                        if k % 4 == 3:
                            kt = m * (ST // 4) + k // 4
                            nm = 2 if kt % 2 == 0 else 3
                            tp = tps_pool.tile(
                                [P, nm, P], BF16, tag="tp", name=f"tp{rep}_{kt}"
                            )
                            for tm in range(nm):
                                nc.tensor.matmul(
                                    tp[:, tm, :],
                                    xs(kt)[:, tm * P : (tm + 1) * P],
                                    idt_b[:],
                                    is_transpose=True,
                                    start=(tm == 0),
                                    stop=(tm == nm - 1),
                                )
                            nc.scalar.copy(xt[:, 0:nm, kt * P : (kt + 1) * P], tp[:])
                    nc.scalar.copy(g_row[m][:, G_START[m] :], g_ps[m][:])
                    for mm, n in G_FIX:
                        if mm != m:
                            continue
                        tfix = tps_pool.tile(
                            [P, P], BF16, tag="tp", name=f"tf{rep}_{mm}{n}"
                        )
                        nc.tensor.matmul(
                            tfix[:],
                            g_row[mm][:, n * P : (n + 1) * P],
                            idt_b[:],
                            is_transpose=True,
                            start=True,
                            stop=True,
                        )
                        nc.scalar.copy(g_row[n][:, mm * P : (mm + 1) * P], tfix[:])

                # With the contiguous layout, o_ps partition c holds output
                # row 32c + i, so DRAM rows for consecutive i are adjacent:
                # pack 2 i-blocks per SBUF tile and DMA 4 KB-contiguous runs.
                out_r = out.rearrange("(c r) d -> c r d", c=P)
                ob = None
                for i in range(ST):
                    o_ps = ops_pool.tile([P, D], F32, tag="o", name=f"ops{rep}_{i}")
                    for dk in range(DT):
                        nc.tensor.matmul(
                            o_ps[:],
                            xt[:, dk, i * P : (i + 1) * P],
                            g_row[dk][:],
                            start=(dk == 0),
                            stop=(dk == DT - 1),
                        )
                    if i % 2 == 0:
                        ob = osb_pool.tile([P, 2, D], F32, tag="ob", name=f"ob{rep}_{i}")
                    nc.scalar.copy(ob[:, i % 2, :], o_ps[:])
                    if i % 2 == 1:
                        nc.sync.dma_start(out_r[:, i - 1 : i + 1, :], ob[:])

    nc.compile()
    return nc


def _get_nc(reps=1):
    key = f"nc{reps}"
    if key not in _cache:
        _cache[key] = _build(reps)
    return _cache[key]


def kernel(inputs: np.ndarray, _reps=1, **run_kwargs) -> np.ndarray:
    nc = _get_nc(_reps)
    ident = np.eye(P, dtype=np.float32)
    in_maps = [
        {"x": np.ascontiguousarray(inputs[b], dtype=np.float32), "ident": ident}
        for b in range(B)
    ]
    res = run_bass_kernel_spmd(nc, in_maps, core_ids=list(range(B)), **run_kwargs)
    _cache["last_result"] = res
    return np.stack([res.results[b]["out"] for b in range(B)], axis=0)



# revision 4
# speedup vs baseline: 1.2893x; 1.2893x over previous
"""Self-contained Trainium2 Bass kernel for per-batch out = X @ (X^T @ X).

Full input: [8, 4096, 512] fp32. Sharding: data-parallel over batch --
core b computes batch element b entirely on its own NeuronCore
(no cross-core communication).

Per-core algorithm (X is [4096, 512], S=4096, D=512), bf16 compute
(inputs are N(0,1); bf16 operands with fp32 PSUM accumulation give
~2e-3 relative error, well inside the 2e-2 gate), fp32 output.
The gram contraction over S is invariant to how S-rows are assigned
to partitions, so SBUF partition p holds rows [32p, 32p+32): input
DMA then reads 8 KB-contiguous runs (1 descriptor per partition per
group) and k-step k contracts rows {32c + k | c}.

  Cast:    X groups are DMA'd in as fp32 (GpSimd-issued so input issue
           never queues behind the sync engine's output-DMA waits) and
           cast to bf16 (xb), one 128-row k-step at a time, split
           ACT/DVE. In steady state the next rep's casts run during
           this rep's phase 2.
  Phase 1: G = X^T @ X -- G is symmetric, so matmuls compute only the
           upper-triangle block-rows: row m covers columns [128m:512]
           (N = 512/384/256/128), contracting over S in 32 k-steps
           into 4 PSUM banks (fp32). The loop is m-major: block-row 0
           finishes first, so its bf16 SBUF copy (one tile per row
           keeps phase-2 deps per-row) and the fixups it feeds happen
           while PE is still on rows 1-3 -- no copy bubble at the
           phase boundary.
  Fixup:   the 6 lower-triangle blocks G[n, 128m:128(m+1)] (n>m) are
           PE-transposes of the computed G[m, 128n:128(n+1)].
  X^T:     xt[:, m, 128k + c] = x[32c + k, 128m + p]. d-blocks 0,1
           (and block 2 for odd k) come from PE transposes (128x128
           bf16, 1 cycle/row) spread across the gram row sections to
           pace the 2-bank PSUM ring against the ACT copy stream.
           d-block 3 (all k) and d-block 2 (even k) come from DVE
           StreamTranspose: one instruction per (output partition
           group a, input partition group b) pair, transposing every
           32x32 block with that (a, b) across all k-tiles in one go
           (StreamTranspose pairs blocks positionally between the
           in/out views, so the partition-group pair is fixed per
           instruction; wide free dims amortize instruction overhead).
           This splits the transpose load across PE and the
           otherwise-idle DVE.
  Phase 2: out = X @ G -- stationary operand is an X^T tile (bf16),
           moving operand is a G block-row [128, 512] bf16,
           accumulating fp32 over 4 d-blocks; one 512-wide ACT copy
           PSUM->SBUF per i. o_ps partition c holds output row
           32c + i, so 2 consecutive i-blocks pack into one SBUF tile
           and DMA out as 4 KB-contiguous runs on the sync queue.

  xt, xb and the G row tiles are double-buffered by rep parity so rep
  r+1's phase 1 (and its input casts/transposes) never waits on rep
  r's phase 2. Steady-state per-rep time is PE-bound at ~48.6 us in
  the CoreSim cost model (PE ~97% busy; DMA floor for the 16 MB of
  fp32 I/O is ~46.6 us).

Known stack hazards worked around here:
  - plain fp32 matmul hangs on HW -> f32r/bf16 operands only.
  - DVE reading more than 512 bytes/partition from PSUM hangs ->
    wide fp32 PSUM reads go to ACT; DVE PSUM reads stay at
    256 bf16 elements (512 B) per instruction.
  - DMA cannot cast -> DRAM inputs are declared float32r (same 4-byte
    layout as fp32; numpy float32 binds unchanged) and cast on-chip.
"""

import sys

sys.path.insert(0, "/opt/trn_rl_repo")

import numpy as np  # noqa: E402
import concourse.bacc as bacc  # noqa: E402
import concourse.mybir as mybir  # noqa: E402
import concourse.tile as tile  # noqa: E402
from concourse.bass_utils import run_bass_kernel_spmd  # noqa: E402

B, S, D = 8, 4096, 512
P = 128
ST = S // P  # 32 s-tiles
DT = D // P  # 4 d-tiles
SG = 4  # s-tiles per input DMA group
F32 = mybir.dt.float32
F32R = mybir.dt.float32r
BF16 = mybir.dt.bfloat16

# Phase-1 (gram) matmul m-block row m covers columns [128m:512].
G_START = [0 * P, 1 * P, 2 * P, 3 * P]
# Lower-triangle blocks reconstructed by transpose: (src_row, dst_row).
G_FIX = [(0, 1), (0, 2), (1, 2), (0, 3), (1, 3), (2, 3)]

_cache: dict = {}


def _build(reps=1):
    nc = bacc.Bacc("TRN2", target_bir_lowering=False, debug=False)
    x = nc.dram_tensor("x", [S, D], F32R, kind="ExternalInput")
    ident = nc.dram_tensor("ident", [P, P], F32R, kind="ExternalInput")
    out = nc.dram_tensor("out", [S, D], F32, kind="ExternalOutput")

    with tile.TileContext(nc) as tc:
        with (
            tc.tile_pool(name="xs", bufs=3) as xs_pool,
            tc.tile_pool(name="persist", bufs=1) as persist,
            tc.tile_pool(name="osb", bufs=4) as osb_pool,
            tc.tile_pool(name="gps", bufs=DT, space="PSUM") as gps_pool,
            tc.tile_pool(name="tps", bufs=2, space="PSUM") as tps_pool,
            tc.tile_pool(name="ops", bufs=2, space="PSUM") as ops_pool,
        ):
            idt = persist.tile([P, P], F32R, tag="ident", name="idt")
            nc.sync.dma_start(idt[:], ident[:])
            idt_b = persist.tile([P, P], BF16, tag="identb", name="idtb")
            nc.vector.tensor_copy(idt_b[:], idt[:])

            # Double-buffered by rep parity:
            # xt[p, m, 128k + c] = x[32c + k, m*128 + p] (bf16)
            # g_row[m][p, e] = gram[m*128 + p, e] (bf16)
            xts = [
                persist.tile([P, DT, S], BF16, tag=f"xt{par}", name=f"xt{par}")
                for par in range(2)
            ]
            # xbs[par][p, k, d] = bf16 x[32p + k, d]: one tile per parity so
            # the DVE StreamTranspose free dims can span all 32 k-tiles.
            xbs = [
                persist.tile([P, ST, D], BF16, tag=f"xb{par}", name=f"xb{par}")
                for par in range(2)
            ]
            g_rows = [
                [
                    persist.tile([P, D], BF16, tag=f"g{par}_{m}", name=f"g{par}_{m}")
                    for m in range(DT)
                ]
                for par in range(2)
            ]

            for rep in range(reps):
                xt = xts[rep % 2]
                xb = xbs[rep % 2]
                g_row = g_rows[rep % 2]

                xg = []
                for j in range(ST // SG):
                    t = xs_pool.tile([P, SG, D], F32R, tag="x", name=f"xg{rep}_{j}")
                    nc.gpsimd.dma_start(
                        t[:],
                        x.rearrange("(p r) d -> p r d", p=P)[
                            :, j * SG : (j + 1) * SG, :
                        ],
                    )
                    xg.append(t)

                for j in range(ST // SG):
                    for n in range(SG):
                        k = j * SG + n
                        if k % 8 < 3:
                            nc.scalar.copy(xb[:, k, :], xg[j][:, n, :])
                        else:
                            nc.vector.tensor_copy(xb[:, k, :], xg[j][:, n, :])

                # DVE StreamTranspose for d-blocks 2,3 of X^T: instruction
                # (m, a, b) transposes every 32x32 block whose output
                # partition group is a and input partition group is b,
                # across all 32 k-tiles at once. Views are (32, k, 32) on
                # both sides so the positional block pairing lines up
                # (StreamTranspose cannot move blocks across partition
                # groups within one instruction).
                xt_k = xt.rearrange("p m (k v) -> p m k v", v=P)
                for m in range(3, 4):
                    for a in range(4):
                        for b in range(4):
                            nc.vector.transpose(
                                xt_k[32 * a : 32 * (a + 1), m, :, 32 * b : 32 * (b + 1)],
                                xb[
                                    32 * b : 32 * (b + 1),
                                    :,
                                    m * P + 32 * a : m * P + 32 * (a + 1),
                                ],
                            )
                # m=2, even k on DVE as well (odd k stays on PE)
                xt_e = xt.rearrange("p m (k e v) -> p m k e v", e=2, v=P)
                xb_e = xb.rearrange("p (k e) d -> p k e d", e=2)
                for a in range(4):
                    for b in range(4):
                        nc.vector.transpose(
                            xt_e[32 * a : 32 * (a + 1), 2, :, 0, 32 * b : 32 * (b + 1)],
                            xb_e[
                                32 * b : 32 * (b + 1),
                                :,
                                0,
                                2 * P + 32 * a : 2 * P + 32 * (a + 1),
                            ],
                        )

                def xs(k):
                    return xb[:, k, :]

                g_ps = [
                    gps_pool.tile(
                        [P, D - G_START[m]], F32, tag="g", name=f"gps{rep}_{m}"
                    )
                    for m in range(DT)
                ]
                # Gram runs m-major: finish G block-row 0 first so its SBUF
                # copy and the fixup transposes it feeds happen while PE is
                # still on rows 1-3 -- by phase 2 every G row is ready and
                # the phase boundary has no copy/fixup bubble. The X^T
                # transpose k-groups are spread across the row sections to
                # pace the PSUM ring against the ACT copy stream.
                for m in range(DT):
                    for k in range(ST):
                        nc.tensor.matmul(
                            g_ps[m][:],
                            xs(k)[:, m * P : (m + 1) * P],
                            xs(k)[:, G_START[m] :],
                            start=(k == 0),
                            stop=(k == ST - 1),
                        )
                        if k % 4 == 3:
                            kt = m * (ST // 4) + k // 4
                            nm = 2 if kt % 2 == 0 else 3
                            tp = tps_pool.tile(
                                [P, nm, P], BF16, tag="tp", name=f"tp{rep}_{kt}"
                            )
                            for tm in range(nm):
                                nc.tensor.matmul(
                                    tp[:, tm, :],
                                    xs(kt)[:, tm * P : (tm + 1) * P],
                                    idt_b[:],
                                    is_transpose=True,
                                    start=(tm == 0),
                                    stop=(tm == nm - 1),
                                )
                            nc.scalar.copy(xt[:, 0:nm, kt * P : (kt + 1) * P], tp[:])
                    nc.scalar.copy(g_row[m][:, G_START[m] :], g_ps[m][:])
                    for mm, n in G_FIX:
                        if mm != m:
                            continue
                        tfix = tps_pool.tile(
                            [P, P], BF16, tag="tp", name=f"tf{rep}_{mm}{n}"
                        )
                        nc.tensor.matmul(
                            tfix[:],
                            g_row[mm][:, n * P : (n + 1) * P],
                            idt_b[:],
                            is_transpose=True,
                            start=True,
                            stop=True,
                        )
                        nc.scalar.copy(g_row[n][:, mm * P : (mm + 1) * P], tfix[:])

                # With the contiguous layout, o_ps partition c holds output
                # row 32c + i, so DRAM rows for consecutive i are adjacent:
                # pack 2 i-blocks per SBUF tile and DMA 4 KB-contiguous runs.
                out_r = out.rearrange("(c r) d -> c r d", c=P)
                ob = None
                for i in range(ST):
                    o_ps = ops_pool.tile([P, D], F32, tag="o", name=f"ops{rep}_{i}")
                    for dk in range(DT):
                        nc.tensor.matmul(
                            o_ps[:],
                            xt[:, dk, i * P : (i + 1) * P],
                            g_row[dk][:],
                            start=(dk == 0),
                            stop=(dk == DT - 1),
                        )
                    if i % 2 == 0:
                        ob = osb_pool.tile([P, 2, D], F32, tag="ob", name=f"ob{rep}_{i}")
                    nc.scalar.copy(ob[:, i % 2, :], o_ps[:])
                    if i % 2 == 1:
                        nc.sync.dma_start(out_r[:, i - 1 : i + 1, :], ob[:])

    nc.compile()
    return nc


def _get_nc(reps=1):
    key = f"nc{reps}"
    if key not in _cache:
        _cache[key] = _build(reps)
    return _cache[key]


def kernel(inputs: np.ndarray, _reps=1, **run_kwargs) -> np.ndarray:
    nc = _get_nc(_reps)
    ident = np.eye(P, dtype=np.float32)
    in_maps = [
        {"x": np.ascontiguousarray(inputs[b], dtype=np.float32), "ident": ident}
        for b in range(B)
    ]
    res = run_bass_kernel_spmd(nc, in_maps, core_ids=list(range(B)), **run_kwargs)
    _cache["last_result"] = res
    return np.stack([res.results[b]["out"] for b in range(B)], axis=0)

